# revision 34
# baseline (speedup 1.0000x reference)
"""Trainium2 Bass kernel for nn_Model_47107201302874.

loss = sum((phi - lam)**2) with phi = kron(v_0..v_25), v_i = [sin|th_i|, cos|th_i|].

Sharding: core d owns the 2^23 lam elements whose top-3 bits equal d.
Locally  phi[p,k,s] = c_d * A[p] * B1[k] * B2[s]  with
  c_d = v_0[b0] v_1[b1] v_2[b2]          (d = b0 b1 b2)
  A   = kron(v_3..v_9)    [128]   (p = bits 3..9)
  B1  = kron(v_10..v_16)  [128]   (k = bits 10..16)
  B2  = kron(v_17..v_25)  [512]   (s = bits 17..25)
Per p-subtile [128(k), 512(s)] one fused custom-DVE op:
  out = (C * A'[p] - lam_p)^2, accum_out -> acc[:, p]   (C = outer(B1,B2))

The kernel is HBM-bound: the lam stream runs at the measured ~340 GB/s
per-core ceiling (~98 us for 32 MiB) on a single HWDGE queue; ring
splitting / bigger chunks / bf16 cast-DMA do not raise it. What the
exec time is actually sensitive to beyond the stream:
  - per-DVE-op semaphore traffic stalls the DMA queue (~45 ns/op), so
    the fused op (one per subtile, no separate square) matters;
  - the serial tail after the last lam byte. The epilogue is split so
    the bulk of acc is reduced and DMA'd out mid-stream (hidden); only
    a raw 4-column DMA of the last p-columns trails the final compute,
    on the otherwise-idle ACT HWDGE queue.
Host sums the [128, 5] partials from all 8 cores in float64.
"""

import os
import sys
from contextlib import ExitStack

import numpy as np

for _p in (
    "/opt/trn_rl_repo",
    "/root/.axon_site/_ro/trn_rl_repo",
    "/root/.axon_site/_ro/pypackages",
):
    if os.path.isdir(_p) and _p not in sys.path:
        sys.path.append(_p)

import concourse.bacc as bacc
import concourse.mybir as mybir
import concourse.tile as tile
from concourse.bass_utils import run_bass_kernel_spmd

F32 = mybir.dt.float32
BF16 = mybir.dt.bfloat16
ALU = mybir.AluOpType
ACTF = mybir.ActivationFunctionType

N = 26
NCORES = 8
P, K, S = 128, 128, 512  # p: bits 3..9, k: bits 10..16, s: bits 17..25
# chunk size 2 (512 KiB DMAs): small steady-state compute lag behind the DMA
# stream while keeping per-op fixed costs amortized.
CHUNKS = [2] * 62 + [1, 1, 1, 1]
assert sum(CHUNKS) == P
LAM_BUFS = 24
# alternate lam chunk DMAs between the two HWDGE rings (sync=SP /
# scalar=ACT): 0 = all sync. On HW the rings drain the same 16 SDMA
# engines but a single ring may not sustain the full HBM rate.
DMA_SPLIT = 0
# debug: skip all per-chunk compute to measure the pure DMA stream rate.
SKIP_COMPUTE = False
# debug: process only every Nth p-subtile (timing probe; wrong result).
COMPUTE_EVERY = 1
# prebuild phi = A[p]*C per chunk on the (otherwise idle) ACT engine and run
# ONE fused DVE op per chunk over the whole [K, cnt, S] tile. Cuts the
# per-op semaphore traffic gating the DMA queue (~45ns/op measured) and
# helps the steady-state loop metric slightly, but costs more in the
# single-shot epilogue structure the harness measures — so off.
PHI_PREBUILD = False
# custom-DVE out placement: 'inplace' writes the squares back over the lam
# tile; 'scratch' writes a dedicated tile (leaves lam tiles read-only);
# 'const' also reads ct for in1 (no lam dependency at all; wrong result,
# timing probe only).
DVE_OUT = "inplace"
# stream lam as bf16 via SWDGE cast-DMA. Measured: no speed gain over the
# fp32 HWDGE stream (the DMA stream runs at full rate either way), so keep
# fp32 for exactness.
LAM_BF16 = False
# columns below TAIL_P are bulk-reduced mid-stream; the rest ship raw in
# the tail. p=123 finishes two chunks before the end (chunks are 2 wide).
TAIL_P = 124
OUT_W = 1 + (P - TAIL_P)
USE_CUSTOM_DVE = True
# loss = sum(lam^2) - 2*sum(phi*lam) + sum(phi^2): ACT squares lam, PE does
# the cross term as PSUM-accumulated matmuls, DVE is nearly idle.
USE_MATMUL = False
# hybrid: even p-subtiles use the fused DVE op; odd ones below PE_LAST use
# the bf16-PE cross-term path. Measured SLOWER than the pure fused-DVE
# path on HW (extra SBUF traffic contends with the DMA stream), so off.
USE_HYBRID = False
PE_LAST = 112
# fraction of sub-tiles diffed on GPSIMD (+ACT square) instead of the DVE
# custom op: 0 = all DVE. With both streams the per-engine busy drops well
# under the DMA floor. 2 = every 2nd sub-tile on POOL.
POOL_EVERY = 0

PI = float(np.pi)

_CACHE = {}


def _register_sqdiff_op():
    """Register a fused DVE op: out = (in0*s0 - in1)^2, accum_out = sum(out).

    One DVE pass replaces the scalar_tensor_tensor + ACT Square pair, so the
    whole reduction runs on the vector engine with no activation stage.
    """
    from operator import add

    from concourse import dve_ops, dve_spec
    from concourse.dve_uop import DveOpSpec

    name = "SQDIFF_ACC_ANT"
    for op in dve_ops.OPS:
        if op.name == name:
            return op

    def ref(in0, in1, c0, c1, c2):
        b = ((in0.astype(np.float32) * c0 - in1) ** 2).astype(np.float32)
        return b, b.reshape(b.shape[0], -1).sum(axis=-1, keepdims=True)

    spec = dve_spec.Spec(
        body=dve_spec.sq(dve_spec.Src0 * dve_spec.C0 - dve_spec.Src1),
        accum=add,
        accum_init=dve_spec.Zero,
        reference=ref,
    )
    row = dve_ops._CUSTOM_DVE_ROW_BASE + len(dve_ops.OPS)
    assert row < 0x20, "custom-DVE opcode rows exhausted"
    dve_ops._SUB_OPCODE_FOR_NAME[name] = row
    shas = {}
    for ver in ("v3", "v4"):
        uops = dve_spec.lower(spec, ver=ver)
        shas[ver] = DveOpSpec(
            name=name, opcode=row, uops=uops, rd1_en=dve_spec._has_src1(spec)
        ).sha(ver)
    op = dve_ops.DveOp(name, spec, subdim=False, uops_sha=shas)
    dve_ops.OPS.append(op)
    dve_ops.CUSTOM_DVE_SPECS[name] = spec
    return op


def _body(
    ctx, tc, out_ap, theta_ap, dbits_ap, lam_ap, reps=1, loop=False, epi_in_loop=False
):
    nc = tc.nc
    const = ctx.enter_context(tc.tile_pool(name="const", bufs=1))
    psum = ctx.enter_context(tc.tile_pool(name="psum", bufs=1, space="PSUM"))
    lam_pool = ctx.enter_context(tc.tile_pool(name="lam", bufs=LAM_BUFS))
    scratch = ctx.enter_context(tc.tile_pool(name="scratch", bufs=3))

    # ---- prologue: per-factor sin/cos ------------------------------------
    # tiny loads go on whichever DGE queue is NOT streaming lam.
    th_eng = nc.sync if LAM_BF16 else nc.gpsimd
    th = const.tile([1, N], F32, tag="th")
    th_eng.dma_start(th[:], theta_ap)
    db = const.tile([1, 3], F32, tag="db")
    th_eng.dma_start(db[:], dbits_ap)

    av = const.tile([1, N], F32, tag="av")
    nc.scalar.activation(av[:], th[:], ACTF.Abs)

    # Sin LUT only valid on [-pi, pi]: wrap x (in [0, 3pi)) to x - 2pi*(x > pi).
    sn = const.tile([1, N], F32, tag="sn")
    cs = const.tile([1, N], F32, tag="cs")
    wa = const.tile([1, N], F32, tag="wa")
    wm = const.tile([1, N], F32, tag="wm")
    for dst, shift in ((sn, 0.0), (cs, PI / 2)):
        # wa = |th| + shift ; wm = (wa > pi) ; wa -= 2pi*wm ; dst = Sin(wa)
        if shift:
            nc.vector.tensor_scalar_add(wa[:], av[:], shift)
        else:
            nc.vector.tensor_copy(wa[:], av[:])
        nc.vector.tensor_scalar(wm[:], wa[:], PI, None, op0=ALU.is_gt)
        nc.vector.scalar_tensor_tensor(
            wa[:], wm[:], -2.0 * PI, wa[:], op0=ALU.mult, op1=ALU.add
        )
        nc.scalar.activation(dst[:], wa[:], ACTF.Sin)

    # c_d = prod_i (sn[i] + dbits[i]*(cs[i]-sn[i])), i<3
    sel = const.tile([1, 3], F32, tag="sel")
    nc.vector.tensor_sub(sel[:], cs[0:1, 0:3], sn[0:1, 0:3])
    nc.vector.tensor_mul(sel[:], sel[:], db[:])
    nc.vector.tensor_add(sel[:], sel[:], sn[0:1, 0:3])
    cd = const.tile([1, 1], F32, tag="cd")
    nc.vector.tensor_mul(cd[:], sel[0:1, 0:1], sel[0:1, 1:2])
    nc.vector.tensor_mul(cd[:], cd[:], sel[0:1, 2:3])

    # ---- kron ladders (free dim of partition 0) --------------------------
    kr_a = const.tile([1, S], F32, tag="kr_a")
    kr_b = const.tile([1, S], F32, tag="kr_b")

    kr_c = const.tile([1, S], F32, tag="kr_c")
    kr_d = const.tile([1, S], F32, tag="kr_d")

    def kron(idxs, seed, bufs, eng):
        # ladder on `eng`: DVE uses tensor_scalar_mul, ACT uses Copy+scale —
        # splitting the chains across engines shortens the DVE prologue.
        cur, other = bufs

        def mul(dst, src, L, sc):
            if eng == "act":
                nc.scalar.activation(dst, src[0:1, 0:L], ACTF.Copy, scale=sc)
            else:
                nc.vector.tensor_scalar_mul(dst, src[0:1, 0:L], sc)

        if seed is None:
            nc.vector.memset(cur[0:1, 0:1], 1.0)
        else:
            nc.vector.tensor_copy(cur[0:1, 0:1], seed)
        L = 1
        for i in idxs:
            d3 = other[0:1, 0 : 2 * L].rearrange("a (l t) -> a l t", t=2)
            mul(d3[:, :, 0], cur, L, sn[0:1, i : i + 1])
            mul(d3[:, :, 1], cur, L, cs[0:1, i : i + 1])
            cur, other = other, cur
            L *= 2
        return cur[0:1, 0:L]

    arow_src = kron(range(3, 10), cd, (kr_a, kr_b), "act")  # [1,128] = c_d*A
    arow = const.tile([1, P], F32, tag="arow")
    nc.vector.tensor_copy(arow[:], arow_src)

    b2row_src = kron(range(17, 26), None, (kr_c, kr_d), "dve")  # [1,512]
    b2row = const.tile([1, S], F32, tag="b2row")
    nc.vector.tensor_copy(b2row[:], b2row_src)
    b1row_src = kron(range(10, 17), None, (kr_c, kr_d), "dve")  # [1,128]
    b1row = const.tile([1, P], F32, tag="b1row")
    nc.vector.tensor_copy(b1row[:], b1row_src)

    ct = arep = None
    if USE_MATMUL or USE_HYBRID:
        # H[k, p] = B1[k] * A'[p]; kept in bf16 — the cross term is O(1)
        # against a ~7e7 loss, so bf16 rounding there is invisible.
        h_ps = psum.tile([P, P], F32, tag="h_ps")
        nc.tensor.matmul(h_ps[:], lhsT=b1row[:], rhs=arow[:], start=True, stop=True)
        hmat = const.tile([P, P], BF16, tag="hmat")
        nc.scalar.copy(hmat[:], h_ps[:])

        # phi2 = sum(phi^2) over the PE-path subtiles:
        # (sum of A'[p]^2 over assigned p) * sum(B1^2) * sum(B2^2)
        phi2 = const.tile([1, 1], F32, tag="phi2")
        p2t = const.tile([1, S], F32, tag="p2t")
        p2s = const.tile([1, 1], F32, tag="p2s")
        nc.vector.memset(phi2[:], 1.0)
        nc.vector.tensor_mul(p2t[0:1, 0:P], arow[:], arow[:])
        asq = p2t[0:1, 1:PE_LAST:2] if USE_HYBRID else p2t[0:1, 0:P]
        nc.vector.tensor_reduce(
            p2s[:], asq, axis=mybir.AxisListType.X, op=ALU.add
        )
        nc.vector.tensor_copy(phi2[:], p2s[:])
        for row, ln in ((b1row, P), (b2row, S)):
            nc.vector.tensor_mul(p2t[0:1, 0:ln], row[0:1, 0:ln], row[0:1, 0:ln])
            nc.vector.tensor_reduce(
                p2s[:], p2t[0:1, 0:ln], axis=mybir.AxisListType.X, op=ALU.add
            )
            nc.vector.tensor_mul(phi2[:], phi2[:], p2s[:])
    if not USE_MATMUL:
        ones_r = const.tile([1, P], F32, tag="ones")
        nc.vector.memset(ones_r[:], 1.0)

        c_ps = psum.tile([P, S], F32, tag="c_ps")
        nc.tensor.matmul(c_ps[:], lhsT=b1row[:], rhs=b2row[:], start=True, stop=True)
        ct = const.tile([P, S], BF16 if LAM_BF16 else F32, tag="ct")
        nc.scalar.copy(ct[:], c_ps[:])

        a_ps = psum.tile([P, P], F32, tag="a_ps")
        nc.tensor.matmul(a_ps[:], lhsT=ones_r[:], rhs=arow[:], start=True, stop=True)
        arep = const.tile([P, P], F32, tag="arep")
        nc.scalar.copy(arep[:], a_ps[:])

    # ---- main loop -------------------------------------------------------
    use_custom = USE_CUSTOM_DVE and not USE_MATMUL
    sqdiff = _register_sqdiff_op() if use_custom else None
    acc = const.tile([P, P if use_custom else len(CHUNKS)], F32, tag="acc")
    if SKIP_COMPUTE or COMPUTE_EVERY > 1:
        nc.vector.memset(acc[:], 0.0)
    lam_r = lam_ap.rearrange("p k s -> k p s")
    if USE_MATMUL or USE_HYBRID:
        w_ps = psum.tile([1, S], F32, tag="w_ps")

    def main_pass():
        p0 = 0
        for t, cnt in enumerate(CHUNKS):
            lt = lam_pool.tile([K, cnt, S], BF16 if LAM_BF16 else F32, tag="lt")
            if LAM_BF16:
                dma_eng = nc.gpsimd  # SWDGE: casts f32 -> bf16 in the datapath
            else:
                dma_eng = nc.scalar if (DMA_SPLIT and t % DMA_SPLIT) else nc.sync
            dma_eng.dma_start(lt[:], lam_r[:, p0 : p0 + cnt, :])
            if SKIP_COMPUTE:
                pass
            elif USE_MATMUL:
                # DVE: bf16 copy of the chunk; PE: w[s] += sum_k H[k,p]*lam[p,k,s]
                ltb = scratch.tile([K, cnt, S], BF16, tag="ltb")
                nc.vector.tensor_copy(
                    ltb[:].rearrange("k a s -> k (a s)"),
                    lt[:].rearrange("k a s -> k (a s)"),
                )
                for j in range(cnt):
                    p = p0 + j
                    nc.tensor.matmul(
                        w_ps[:],
                        lhsT=hmat[:, p : p + 1],
                        rhs=ltb[:, j, :],
                        start=(p == 0),
                        stop=(p == P - 1),
                    )
                # ACT: acc[:, t] = sum(lam^2) for this chunk (to scratch)
                sq = scratch.tile([K, cnt, S], F32, tag="sqout")
                nc.scalar.activation(
                    sq[:].rearrange("k a s -> k (a s)"),
                    lt[:].rearrange("k a s -> k (a s)"),
                    ACTF.Square,
                    accum_out=acc[:, t : t + 1],
                )
            elif use_custom and PHI_PREBUILD:
                pt = scratch.tile([K, cnt, S], F32, tag="pt")
                for j in range(cnt):
                    p = p0 + j
                    nc.scalar.activation(
                        pt[:, j, :], ct[:], ACTF.Copy, scale=arep[:, p : p + 1]
                    )
                nc.vector._custom_dve(
                    sqdiff,
                    out=pt[:],
                    in0=lt[:],
                    in1=pt[:],
                    s0=1.0,
                    accum_out=acc[:, t : t + 1],
                )
            elif use_custom:
                for j in range(cnt):
                    p = p0 + j
                    if COMPUTE_EVERY > 1 and p % COMPUTE_EVERY:
                        continue
                    sl = lt[:, j, :]
                    if USE_HYBRID and p % 2 == 1 and p < PE_LAST:
                        # PE path: bf16 cast (DVE 2x) + cross-term matmul;
                        # ACT squares the fp32 subtile for the lam^2 term.
                        ltb = scratch.tile([K, S], BF16, tag="ltb")
                        nc.vector.tensor_copy(ltb[:], sl)
                        nc.tensor.matmul(
                            w_ps[:],
                            lhsT=hmat[:, p : p + 1],
                            rhs=ltb[:],
                            start=(p == 1),
                            stop=(p == PE_LAST - 1),
                        )
                        sq = scratch.tile([K, S], F32, tag="sqout")
                        nc.scalar.activation(
                            sq[:], sl, ACTF.Square, accum_out=acc[:, p : p + 1]
                        )
                    elif POOL_EVERY and p % POOL_EVERY == (POOL_EVERY - 1):
                        # ACT: phi = C * A'[p]; POOL: sl -= phi; ACT: square+acc
                        phi = scratch.tile([K, S], F32, tag="phi")
                        nc.scalar.activation(
                            phi[:], ct[:], ACTF.Copy, scale=arep[:, p : p + 1]
                        )
                        nc.gpsimd.tensor_tensor(sl, sl, phi[:], op=ALU.subtract)
                        nc.scalar.activation(
                            sl, sl, ACTF.Square, accum_out=acc[:, p : p + 1]
                        )
                    else:
                        if DVE_OUT == "inplace":
                            o, i1 = sl, sl
                        else:
                            dveout = scratch.tile([K, S], F32, tag="dveout")
                            o = dveout[:]
                            # 'const': timing probe, no lam dependency
                            i1 = sl if DVE_OUT == "scratch" else ct[:]
                        nc.vector._custom_dve(
                            sqdiff,
                            out=o,
                            in0=ct[:],
                            in1=i1,
                            s0=arep[:, p : p + 1],
                            accum_out=acc[:, p : p + 1],
                        )
            else:
                for j in range(cnt):
                    p = p0 + j
                    sl = lt[:, j, :]
                    nc.vector.scalar_tensor_tensor(
                        sl, ct[:], arep[:, p : p + 1], sl,
                        op0=ALU.mult, op1=ALU.subtract,
                    )
                flat = lt[:].rearrange("k a s -> k (a s)")
                nc.scalar.activation(
                    flat, flat, ACTF.Square, accum_out=acc[:, t : t + 1]
                )
            p0 += cnt

    # ---- epilogue --------------------------------------------------------
    # Split the output: the bulk reduce (cols < TAIL_P) and its DMA complete
    # mid-stream, fully hidden; only a raw 4-column DMA of the last columns
    # sits in the serial tail after the final chunk's compute. Both go on
    # the ACT HWDGE queue, which is idle in the pure-DVE config. The host
    # sums all output elements, so no final combine is needed on-device.
    def epilogue_split():
        ncols = len(CHUNKS) if PHI_PREBUILD else P
        bulk = ncols - (OUT_W - 1)
        rsum = const.tile([P, 1], F32, tag="rsum")
        nc.vector.tensor_reduce(
            rsum[:], acc[:, :bulk], axis=mybir.AxisListType.X, op=ALU.add
        )
        # keep the out DMAs off the ACT queue when it is the phi builder —
        # behind 128 phi ops in FIFO order the bulk DMA would lose its
        # mid-stream overlap.
        q = nc.sync if PHI_PREBUILD else nc.scalar
        q.dma_start(out_ap[:, 0:1], rsum[:])
        q.dma_start(out_ap[:, 1:OUT_W], acc[:, bulk:ncols])

    def epilogue():
        if use_custom and not SKIP_COMPUTE:
            epilogue_split()
            return
        rsum = const.tile([P, 1], F32, tag="rsum")
        nc.vector.tensor_reduce(
            rsum[:], acc[:, : P if use_custom else len(CHUNKS)],
            axis=mybir.AxisListType.X, op=ALU.add,
        )
        if (USE_MATMUL or USE_HYBRID) and not SKIP_COMPUTE:
            # loss_local = sum(lam^2) - 2*cross + phi2 ; fold into rsum[0]
            wrow = const.tile([1, S], F32, tag="wrow")
            nc.scalar.copy(wrow[:], w_ps[:])
            cm = const.tile([1, S], F32, tag="cm")
            nc.vector.tensor_mul(cm[:], wrow[:], b2row[:])
            cross = const.tile([1, 1], F32, tag="cross")
            nc.vector.tensor_reduce(
                cross[:], cm[:], axis=mybir.AxisListType.X, op=ALU.add
            )
            extra = const.tile([1, 1], F32, tag="extra")
            nc.vector.scalar_tensor_tensor(
                extra[:], cross[:], -2.0, phi2[:], op0=ALU.mult, op1=ALU.add
            )
            nc.vector.tensor_add(rsum[0:1, 0:1], rsum[0:1, 0:1], extra[:])
        nc.sync.dma_start(out_ap[:, 0:1], rsum[:])

    if loop and reps > 1:
        with tc.For_i(0, reps, 1):
            main_pass()
            if epi_in_loop:
                epilogue()
        if not epi_in_loop:
            epilogue()
    else:
        for _rep in range(reps):
            main_pass()
        epilogue()


def build_nc(reps=1, loop=False, epi_in_loop=False):
    key = ("nc", reps, loop, epi_in_loop)
    if key in _CACHE:
        return _CACHE[key]
    nc = bacc.Bacc(
        "TRN2", target_bir_lowering=False, debug=False, num_devices=NCORES
    )
    theta_ap = nc.dram_tensor("theta", [1, N], F32, kind="ExternalInput").ap()
    dbits_ap = nc.dram_tensor("dbits", [1, 3], F32, kind="ExternalInput").ap()
    lam_ap = nc.dram_tensor("lam", [P, K, S], F32, kind="ExternalInput").ap()
    out_ap = nc.dram_tensor("partial", [P, OUT_W], F32, kind="ExternalOutput").ap()
    with tile.TileContext(nc) as tc, ExitStack() as ctx:
        _body(
            ctx, tc, out_ap, theta_ap, dbits_ap, lam_ap,
            reps=reps, loop=loop, epi_in_loop=epi_in_loop,
        )
    nc.compile()
    _CACHE[key] = nc
    return nc


def make_in_maps(theta, lam):
    theta = np.ascontiguousarray(np.asarray(theta, dtype=np.float32)).reshape(1, N)
    lam = np.ascontiguousarray(np.asarray(lam, dtype=np.float32)).reshape(
        NCORES, P, K, S
    )
    in_maps = []
    for d in range(NCORES):
        bits = np.array(
            [[(d >> 2) & 1, (d >> 1) & 1, d & 1]], dtype=np.float32
        )
        in_maps.append({"theta": theta, "dbits": bits, "lam": lam[d]})
    return in_maps


def run(theta, lam, trace=False, **kwargs):
    nc = build_nc()
    in_maps = make_in_maps(theta, lam)
    res = run_bass_kernel_spmd(
        nc, in_maps, list(range(NCORES)), trace=trace, **kwargs
    )
    total = np.float64(0.0)
    for r in res.results:
        total += r["partial"].astype(np.float64).sum()
    return np.array(np.float32(total)), res


def kernel(theta, lam):
    out, _ = run(theta, lam)
    return out

